# revision 32
# baseline (speedup 1.0000x reference)
"""GATv2 2-layer GNN on 8 Trainium2 NeuronCores (self-contained).

Sharding: destination nodes (and their incident edges) are partitioned
across the 8 cores; weights replicated. The halo exchange of source-node
features is done at input-distribution time: for each core the host packs
x[src_e] for its edge slots (transposed, bf16) and ships it as a kernel
input, along with host-built one-hot scatter/expand matrices (Q/QT) for
each 128-edge tile. On device, per dst-block of 128 nodes:
  - xl_e = x_srcT.T @ Wl per 128-edge tile (PE), u = QT.T @ xr + xl in
    PSUM (PE accumulate), leaky-relu on ScalarE, attention logits on
    VectorE (bf16 fast path), exp on ScalarE, softmax numerator and
    denominator accumulated into PSUM via the Q one-hot matmul.
  - Softmax max-subtraction is skipped: logits are O(1) by construction
    so exp() cannot overflow, and softmax is shift-invariant.
  - Per-block epilogue is a single PSUM->SBUF copy into a staging tile;
    softmax normalization + bias + layernorm (+ELU for layer 1) run as
    one batched pass over all blocks at the end (2 act-table loads total).
The h1 exchange between the two layers is done on the host (all-gather
of the 8 per-core slices, then re-pack of h1[src] per edge slot).
"""
import os
import sys
import numpy as np

sys.path.insert(0, "/opt/trn_rl_repo")

import ml_dtypes
import concourse.bacc as bacc
import concourse.mybir as mybir
from concourse.tile import TileContext
from concourse.bass_utils import run_bass_kernel_spmd

dt = mybir.dt
A = mybir.ActivationFunctionType
Op = mybir.AluOpType

N, E = 50000, 800000
F_IN = 128
F_OUT1, H1 = 128, 8
F_OUT2, H2 = 64, 1
NEG_SLOPE = 0.2
LN_EPS = 1e-5
N_CORES = 8
BLK = 128
G = 4
EP_CHUNK = 12

LAST_EXEC_NS = {}


# ---------------------------------------------------------------- host prep
def _host_prep(edge_index):
    src = np.asarray(edge_index[0], dtype=np.int64)
    dst = np.asarray(edge_index[1], dtype=np.int64)
    S = N // N_CORES
    nb = (S + BLK - 1) // BLK

    order = np.argsort(dst, kind="stable")
    src_s, dst_s = src[order], dst[order]
    core_of = dst_s // S

    per_core = []
    counts = np.zeros((N_CORES, nb), dtype=np.int64)
    for c in range(N_CORES):
        m = core_of == c
        sc, dc = src_s[m], dst_s[m] - c * S
        b_of = dc // BLK
        counts[c] = np.bincount(b_of, minlength=nb)
        per_core.append((sc, dc, b_of))

    cbk = [max(1, int(np.max((counts[:, b] + BLK - 1) // BLK))) for b in range(nb)]
    offC = np.concatenate([[0], np.cumsum(cbk)])
    C_total = int(offC[-1])

    src_slots = np.zeros((N_CORES, C_total * 128), dtype=np.int64)
    dl_all = np.full((N_CORES, C_total * 128), -1, dtype=np.int32)
    for c in range(N_CORES):
        sc, dc, b_of = per_core[c]
        for b in range(nb):
            mb = b_of == b
            n = int(counts[c, b])
            o = offC[b] * 128
            src_slots[c, o:o + n] = sc[mb]
            dl_all[c, o:o + n] = dc[mb] - b * BLK

    ar = np.arange(128, dtype=np.int32)
    qmat = np.zeros((N_CORES, 128, C_total, 256), dtype=ml_dtypes.bfloat16)
    for c in range(N_CORES):
        dl_r = dl_all[c].reshape(C_total, 128)
        qmat[c, :, :, 0:128] = (ar[:, None, None] == dl_r[None, :, :])
        qmat[c, :, :, 128:256] = (dl_r.T[:, :, None] == ar[None, None, :])

    return dict(cbk=cbk, offC=offC, C_total=C_total, nb=nb, S=S,
                src_slots=src_slots, qmat=qmat)


def _pack_srcT(x_full, src_slots):
    # [C*128, 128] gather -> transpose -> bf16 [128, C*128]
    xg = np.asarray(x_full, np.float32)[src_slots]
    return np.ascontiguousarray(xg.T).astype(ml_dtypes.bfloat16)


def _pack_sliceT(x_full, c, S, ns_pad):
    sl = np.zeros((ns_pad, x_full.shape[1]), dtype=np.float32)
    sl[:S] = np.asarray(x_full, np.float32)[c * S:(c + 1) * S]
    return np.ascontiguousarray(sl.T).astype(ml_dtypes.bfloat16)


# ---------------------------------------------------------------- builder
def _build_layer(meta, F_out, H, layer, skip_bias=False, skip_g=False,
                 skip_b=False):
    nb, S = meta["nb"], meta["S"]
    cbk, offC, C_total = meta["cbk"], meta["offC"], meta["C_total"]
    C = F_out // H
    ns_tiles = (S + 127) // 128

    nc = bacc.Bacc("TRN2", target_bir_lowering=False, debug=False,
                   num_devices=N_CORES)
    xsrcT = nc.dram_tensor("xsrcT", [128, C_total, 128], dt.bfloat16, kind="ExternalInput").ap()
    qmat = nc.dram_tensor("qmat", [128, C_total, 256], dt.bfloat16, kind="ExternalInput").ap()
    xTs = nc.dram_tensor("xTs", [128, ns_tiles * 128], dt.bfloat16, kind="ExternalInput").ap()
    wl = nc.dram_tensor("wl", [128, F_out], dt.bfloat16, kind="ExternalInput").ap()
    wr = nc.dram_tensor("wr", [128, F_out], dt.bfloat16, kind="ExternalInput").ap()
    att_in = nc.dram_tensor("att", [128, F_out], dt.bfloat16, kind="ExternalInput").ap()
    bias_in = nc.dram_tensor("bias", [128, F_out], dt.float32, kind="ExternalInput").ap()
    g_in = nc.dram_tensor("g", [128, F_out], dt.float32, kind="ExternalInput").ap()
    b_in = nc.dram_tensor("b", [128, F_out], dt.float32, kind="ExternalInput").ap()
    hout = nc.dram_tensor("hout", [ns_tiles * 128, F_out], dt.float32, kind="ExternalOutput").ap()

    with TileContext(nc) as tc:
        with (
            tc.tile_pool(name="con", bufs=1) as con,
            tc.tile_pool(name="xin", bufs=4) as xin,
            tc.tile_pool(name="ck", bufs=4) as ck,
            tc.tile_pool(name="ep", bufs=2) as ep,
            tc.tile_pool(name="ps_u", bufs=6, space="PSUM") as ps_u,
            tc.tile_pool(name="ps_acc", bufs=2, space="PSUM") as ps_acc,
        ):
            # constants
            wl_sb = con.tile([128, F_out], dt.bfloat16)
            nc.sync.dma_start(out=wl_sb[:], in_=wl[:])
            wr_sb = con.tile([128, F_out], dt.bfloat16)
            nc.sync.dma_start(out=wr_sb[:], in_=wr[:])
            att_sb = con.tile([128, F_out], dt.bfloat16)
            nc.sync.dma_start(out=att_sb[:], in_=att_in[:])
            bias_sb = con.tile([128, F_out], dt.float32)
            nc.sync.dma_start(out=bias_sb[:], in_=bias_in[:])
            g_sb = con.tile([128, F_out], dt.float32)
            nc.sync.dma_start(out=g_sb[:], in_=g_in[:])
            b_sb = con.tile([128, F_out], dt.float32)
            nc.sync.dma_start(out=b_sb[:], in_=b_in[:])

            # dense: XR for own slice (resident), xTs loaded in one DMA
            ctx_dense = nc.named_scope("dense"); ctx_dense.__enter__()
            xTs_sb = con.tile([128, ns_tiles * 128], dt.bfloat16)
            nc.sync.dma_start(out=xTs_sb[:], in_=xTs[:])
            xr_sb = con.tile([128, ns_tiles, F_out], dt.bfloat16)
            for t in range(ns_tiles):
                pd = ps_u.tile([128, G, F_out], dt.float32, tag="u")
                nc.tensor.matmul(pd[:, 0, :], xTs_sb[:, t * 128:(t + 1) * 128],
                                 wr_sb[:], start=True, stop=True)
                if t % 2 == 0:
                    nc.scalar.activation(xr_sb[:, t, :], pd[:, 0, :], A.Copy)
                else:
                    nc.vector.tensor_copy(xr_sb[:, t, :], pd[:, 0, :])
            ctx_dense.__exit__(None, None, None)

            # per-chunk staging buffers for softmax sums [denom | numer]
            sizes = []
            rem = nb
            while rem > 0:
                if rem <= EP_CHUNK:      # taper the tail chunk
                    sizes += [rem - 4, 4] if rem > 5 else [rem]
                    rem = 0
                else:
                    sizes.append(EP_CHUNK)
                    rem -= EP_CHUNK
            bounds = np.concatenate([[0], np.cumsum(sizes)]).astype(int)
            chunks = [(int(bounds[i]), int(bounds[i + 1]))
                      for i in range(len(sizes))]
            stages = [con.tile([128, c1 - c0, H + F_out], dt.float32,
                               name=f"stage{c0}", tag=f"stage{c0}")
                      for c0, c1 in chunks]

            def epilogue(ci):
                """Softmax normalize + (-xr) + bias + LN (+ELU) for one chunk.

                Uses sum(alpha)=1: sum_e alpha_e*xl_e = sum_e alpha_e*u_e - xr,
                so the numerator accumulates ea*u and xr is subtracted here.
                Yields between ops so the caller can interleave the serial
                chain with edge-phase work (in-order engine queues).
                """
                c0, c1 = chunks[ci]
                nbc = c1 - c0
                stage = stages[ci]
                hview = stage[:, :, H:]
                inv = ep.tile([128, EP_CHUNK, H], dt.float32, tag="inv")
                nc.vector.tensor_scalar(inv[:, :nbc, :], stage[:, :, 0:H],
                                        1e-16, None, op0=Op.add)
                yield
                nc.vector.reciprocal(inv[:, :nbc, :], inv[:, :nbc, :])
                yield
                nc.vector.tensor_tensor(
                    hview.rearrange("p b (h c) -> p b h c", h=H),
                    hview.rearrange("p b (h c) -> p b h c", h=H),
                    inv[:, :nbc, :].rearrange("p b (h o) -> p b h o", o=1)
                    .to_broadcast([128, nbc, H, C]),
                    op=Op.mult)
                yield
                nc.vector.tensor_tensor(hview, hview, xr_sb[:, c0:c1, :],
                                        op=Op.subtract)
                yield
                if not skip_bias:
                    nc.vector.tensor_tensor(
                        hview, hview,
                        bias_sb[:].rearrange("p (o f) -> p o f", o=1)
                        .to_broadcast([128, nbc, F_out]),
                        op=Op.add)
                    yield
                mu = ep.tile([128, EP_CHUNK], dt.float32, tag="mu")
                nc.vector.tensor_reduce(mu[:, :nbc], hview,
                                        axis=mybir.AxisListType.X, op=Op.add)
                yield
                nc.vector.tensor_scalar(mu[:, :nbc], mu[:, :nbc], 1.0 / F_out,
                                        None, op0=Op.mult)
                yield
                nc.vector.tensor_tensor(
                    hview, hview,
                    mu[:, :nbc].rearrange("p (b o) -> p b o", o=1)
                    .to_broadcast([128, nbc, F_out]),
                    op=Op.subtract)
                yield
                sq = ep.tile([128, EP_CHUNK, F_out], dt.float32, tag="sq")
                nc.vector.tensor_tensor(sq[:, :nbc, :], hview, hview, op=Op.mult)
                yield
                ss = ep.tile([128, EP_CHUNK], dt.float32, tag="ss")
                nc.vector.tensor_reduce(ss[:, :nbc], sq[:, :nbc, :],
                                        axis=mybir.AxisListType.X, op=Op.add)
                yield
                nc.vector.tensor_scalar(ss[:, :nbc], ss[:, :nbc], 1.0 / F_out,
                                        LN_EPS, op0=Op.mult, op1=Op.add)
                yield
                nc.vector.reciprocal(ss[:, :nbc], ss[:, :nbc])
                yield
                rstd = ep.tile([128, EP_CHUNK], dt.float32, tag="rstd")
                nc.scalar.activation(rstd[:, :nbc], ss[:, :nbc], A.Sqrt)
                yield
                nc.vector.tensor_tensor(
                    hview, hview,
                    rstd[:, :nbc].rearrange("p (b o) -> p b o", o=1)
                    .to_broadcast([128, nbc, F_out]),
                    op=Op.mult)
                yield
                if not skip_g:
                    nc.vector.tensor_tensor(
                        hview, hview,
                        g_sb[:].rearrange("p (o f) -> p o f", o=1)
                        .to_broadcast([128, nbc, F_out]),
                        op=Op.mult)
                    yield
                if not skip_b:
                    nc.vector.tensor_tensor(
                        hview, hview,
                        b_sb[:].rearrange("p (o f) -> p o f", o=1)
                        .to_broadcast([128, nbc, F_out]),
                        op=Op.add)
                    yield
                if layer == 1:
                    m0 = ep.tile([128, EP_CHUNK, F_out], dt.float32, tag="m0")
                    nc.vector.tensor_scalar(m0[:, :nbc, :], hview, 0.0, None,
                                            op0=Op.min)
                    yield
                    ex = ep.tile([128, EP_CHUNK, F_out], dt.float32, tag="ex")
                    nc.scalar.activation(ex[:, :nbc, :], m0[:, :nbc, :], A.Exp)
                    yield
                    nc.vector.scalar_tensor_tensor(hview, ex[:, :nbc, :], -1.0,
                                                   hview, op0=Op.add, op1=Op.max)
                    yield
                nc.sync.dma_start(
                    out=hout[c0 * 128:c1 * 128, :]
                    .rearrange("(b p) f -> p b f", p=128),
                    in_=hview)

            # edge phase as a depth-5 software pipeline over 128-edge groups:
            #   A: u matmuls (PE)        -> P: prelu (ACT, lag 1)
            #   -> M: att mult (GpSimd, lag 2) -> R: head reduce + exp
            #   (DVE+ACT, lag 3) -> X: eav mult + scatter matmuls + block
            #   copy-out (DVE+PE+ACT, lag 4).
            # Every cross-engine dependency is >= 1 iteration old when its
            # consumer is emitted, so the in-order engine queues never stall
            # on a same-iteration handoff. Epilogue chains (per 12-block
            # chunk) are drip-fed 2 ops per new block, mostly on GpSimd.
            pending = []

            def emit_P(st):
                st["lr"] = ck.tile([128, G, F_out], dt.bfloat16, name="lr", tag="lr")
                nc.scalar.activation(st["lr"][:, :st["g"], :],
                                     st["u_ps"][:, :st["g"], :], A.Prelu,
                                     alpha=NEG_SLOPE)

            def emit_M(st):
                g = st["g"]
                st["am"] = ck.tile([128, G, F_out], dt.bfloat16, name="am", tag="am")
                nc.vector.tensor_tensor(
                    st["am"][:, :g, :], st["lr"][:, :g, :],
                    att_sb[:].rearrange("p (o f) -> p o f", o=1)
                    .to_broadcast([128, g, F_out]),
                    op=Op.mult)

            def emit_R(st):
                g = st["g"]
                av = ck.tile([128, G, H], dt.bfloat16, tag="av")
                with nc.allow_low_precision("softmax logits, 16 partial sums"):
                    nc.vector.tensor_reduce(
                        av[:, :g, :].rearrange("p g h -> p (g h)"),
                        st["am"][:, :g, :].rearrange("p g (h c) -> p (g h) c", h=H),
                        axis=mybir.AxisListType.X, op=Op.add)
                st["eav"] = ck.tile([128, G, H + F_out], dt.bfloat16, name="eav", tag="eav")
                nc.scalar.activation(st["eav"][:, :g, 0:H], av[:, :g, :], A.Exp)

            def emit_X(st):
                g, k0, cb, b = st["g"], st["k0"], st["cb"], st["b"]
                eav = st["eav"]
                nc.vector.tensor_tensor(
                    eav[:, :g, H:].rearrange("p g (h c) -> p g h c", h=H),
                    st["u_ps"][:, :g, :].rearrange("p g (h c) -> p g h c", h=H),
                    eav[:, :g, 0:H].rearrange("p g (h o) -> p g h o", o=1)
                    .to_broadcast([128, g, H, C]),
                    op=Op.mult)
                for j in range(g):
                    k = k0 + j
                    nc.tensor.matmul(st["so_ps"][:], st["qm"][:, k, 128:256],
                                     eav[:, j, :], start=(k == 0),
                                     stop=(k == cb - 1))
                if k0 + g == cb:  # block complete: copy out + queue epilogue
                    ci = next(i for i, (c0, c1) in enumerate(chunks)
                              if c0 <= b < c1)
                    nc.scalar.activation(stages[ci][:, b - chunks[ci][0], :],
                                         st["so_ps"][:], A.Copy)
                    if b == chunks[ci][1] - 1:
                        pending.append(epilogue(ci))

            lags = [None, None, None, None]  # [P-wait, M-wait, R-wait, X-wait]

            def step(cur):
                if lags[3] is not None:
                    emit_X(lags[3])
                if lags[0] is not None:
                    emit_P(lags[0])
                if lags[1] is not None:
                    emit_M(lags[1])
                if lags[2] is not None:
                    emit_R(lags[2])
                lags[3], lags[2], lags[1], lags[0] = lags[2], lags[1], lags[0], cur

            def drip():
                for _ in range(2):
                    if pending:
                        try:
                            next(pending[0])
                        except StopIteration:
                            pending.pop(0)

            ctx_edge = nc.named_scope("edge"); ctx_edge.__enter__()
            for b in range(nb):
                cb = cbk[b]
                xs = xin.tile([128, cb, 128], dt.bfloat16, tag="xs")
                nc.sync.dma_start(out=xs[:], in_=xsrcT[:, offC[b]:offC[b] + cb, :])
                qm = xin.tile([128, cb, 256], dt.bfloat16, tag="qm")
                nc.sync.dma_start(out=qm[:], in_=qmat[:, offC[b]:offC[b] + cb, :])

                so_ps = ps_acc.tile([128, H + F_out], dt.float32, tag="so")

                for k0 in range(0, cb, G):
                    g = min(G, cb - k0)
                    u_ps = ps_u.tile([128, G, F_out], dt.float32, tag="u")
                    for j in range(g):
                        k = k0 + j
                        nc.tensor.matmul(u_ps[:, j, :], qm[:, k, 0:128],
                                         xr_sb[:, b, :], start=True, stop=False)
                        nc.tensor.matmul(u_ps[:, j, :], xs[:, k, :], wl_sb[:],
                                         start=False, stop=True)
                    step(dict(qm=qm, k0=k0, g=g, cb=cb, b=b, so_ps=so_ps,
                              u_ps=u_ps))
                drip()
            for _ in range(4):
                step(None)
            while pending:
                try:
                    next(pending[0])
                except StopIteration:
                    pending.pop(0)
            ctx_edge.__exit__(None, None, None)
    nc.compile()
    return nc


def _make_in_maps(meta, x_full, W_l, W_r, att, bias, g_ln, b_ln, F_out):
    S = meta["S"]
    ns_pad = ((S + 127) // 128) * 128
    rep = lambda v: np.tile(np.asarray(v, np.float32).reshape(1, F_out), (128, 1))
    att_rep = rep(att).astype(ml_dtypes.bfloat16)
    bias_rep, g_rep, b_rep = rep(bias), rep(g_ln), rep(b_ln)
    wl_b = np.asarray(W_l, np.float32).astype(ml_dtypes.bfloat16)
    wr_b = np.asarray(W_r, np.float32).astype(ml_dtypes.bfloat16)
    maps = []
    for c in range(N_CORES):
        maps.append({
            "xsrcT": _pack_srcT(x_full, meta["src_slots"][c]).reshape(
                128, meta["C_total"], 128),
            "qmat": meta["qmat"][c],
            "xTs": _pack_sliceT(x_full, c, S, ns_pad),
            "wl": wl_b, "wr": wr_b, "att": att_rep, "bias": bias_rep,
            "g": g_rep, "b": b_rep,
        })
    return maps


def _maybe_install_ntff_hook():
    try:
        import types
        import antenv
        if "antenv.axon_hooks" in sys.modules:
            return True
        mod = types.ModuleType("antenv.axon_hooks")
        state = {"hook": None}
        mod.set_axon_ntff_profile_hook = lambda h: state.__setitem__("hook", h)
        mod.get_axon_ntff_profile_hook = lambda: state["hook"]
        sys.modules["antenv.axon_hooks"] = mod
        antenv.axon_hooks = mod
        from trn_agent_boot.trn_boot import _ntff_profile_via_ctypes
        mod.set_axon_ntff_profile_hook(
            _ntff_profile_via_ctypes("/opt/axon/libaxon_pjrt.so"))
        return True
    except Exception:
        return False


def _run_with_retry(nc, maps, core_ids, trace, tries=3):
    last = None
    for i in range(tries):
        try:
            return run_bass_kernel_spmd(nc, maps, core_ids, trace=trace)
        except Exception as e:  # device flake: retry (fresh exec usually recovers)
            last = e
            if i == tries - 1:
                raise
    raise last


def kernel(**inputs):
    global LAST_EXEC_NS
    LAST_EXEC_NS = {}
    trace = os.environ.get("GAT_TRACE", "0") == "1"
    if trace:
        trace = _maybe_install_ntff_hook()

    x = np.asarray(inputs["x"], np.float32)
    edge_index = np.asarray(inputs["edge_index"])
    meta = _host_prep(edge_index)
    S = meta["S"]
    core_ids = list(range(N_CORES))

    def triv(v, val):
        return bool(np.all(np.asarray(v, np.float32) == val))

    # ---- layer 1
    nc1 = _build_layer(meta, F_OUT1, H1, layer=1,
                       skip_bias=triv(inputs["bias1"], 0.0),
                       skip_g=triv(inputs["g1"], 1.0),
                       skip_b=triv(inputs["b1"], 0.0))
    maps1 = _make_in_maps(meta, x, inputs["Wl1"], inputs["Wr1"],
                          np.asarray(inputs["att1"], np.float32).reshape(-1),
                          inputs["bias1"], inputs["g1"], inputs["b1"], F_OUT1)
    res1 = _run_with_retry(nc1, maps1, core_ids, trace)
    h1 = np.concatenate([res1.results[c]["hout"][:S] for c in range(N_CORES)], axis=0)
    if trace:
        LAST_EXEC_NS["layer1"] = res1.exec_time_ns

    # ---- layer 2
    nc2 = _build_layer(meta, F_OUT2, H2, layer=2,
                       skip_bias=triv(inputs["bias2"], 0.0),
                       skip_g=triv(inputs["g2"], 1.0),
                       skip_b=triv(inputs["b2"], 0.0))
    maps2 = _make_in_maps(meta, h1, inputs["Wl2"], inputs["Wr2"],
                          np.asarray(inputs["att2"], np.float32).reshape(-1),
                          inputs["bias2"], inputs["g2"], inputs["b2"], F_OUT2)
    res2 = _run_with_retry(nc2, maps2, core_ids, trace)
    out = np.concatenate([res2.results[c]["hout"][:S] for c in range(N_CORES)], axis=0)
    if trace:
        LAST_EXEC_NS["layer2"] = res2.exec_time_ns
    return out.astype(np.float32)
